# revision 27
# baseline (speedup 1.0000x reference)
"""Raw-Bass (no TileContext) version of the CudaTensorProduct kernel.

Same v5 pipeline as kernel.py, but with hand-rolled counting semaphores so
the tile framework's fixed ~10us teardown (254 single-semaphore clears +
drain ceremony) is replaced by one barrier + a handful of clears.

Counting semaphores (all monotone within one execution):
  sIN[chunk]    : +16 at each input-DMA chunk's completion (one semaphore
                  per chunk: the 16 parallel DMA engines can complete
                  transfers out of order even within one queue)
  sGPS          : +1 after the warmup-tile memset
  sPE           : +1 per REAL PE matmul (R3: 1..4, main item k: 5+2k, 6+2k)
  sACT          : +1 per ACT cast (R3 halves: 1,2; then out-casts 3..14)
  sDVE          : +1 per DVE op (muls 1..16, tile-4/5 out-casts 17..20)
  sOUT          : +16 per output-DMA completion (final drain waits 256)
"""

import os
import sys
from contextlib import ExitStack

import numpy as np
import ml_dtypes

sys.path.insert(0, "/opt/trn_rl_repo")

import concourse.bass as bass
import concourse.mybir as mybir
from concourse import bacc
from concourse.bass_utils import run_bass_kernel_spmd

N_CORES = 8
B = 16384
BC = B // N_CORES
D1 = 32
DOUT = 1024
F32 = mybir.dt.float32
BF16 = mybir.dt.bfloat16
LS = [0, 1, 2, 3, 0, 1, 2, 3]

N_WARM = 10
CH = 512
H = 1024

# main/mul item order (q, h); identical for both streams
M_ORDER = [(0, 0), (1, 0), (2, 0), (3, 0), (6, 0), (0, 1),
           (7, 0), (1, 1), (2, 1), (3, 1), (6, 1), (7, 1),
           (4, 0), (5, 0), (4, 1), (5, 1)]
Q_AB = {q: (q // 2, q % 2) for q in range(8)}
O_CAST = {(q, h): 'a' for q in range(8) for h in range(2)}
O_CAST.update({(4, 0): 'v', (4, 1): 'v', (5, 0): 'v', (5, 1): 'v'})
# DVE emission program: all 16 muls first, then the tile-4/5 out-casts
# (interleaving casts earlier stalls the DVE queue on PE progress).
DVE_PROG = ([('m', i) for i in range(16)]
            + [('c', 12), ('c', 13), ('c', 14), ('c', 15)])
# fillers emitted before main item k
FILL = {0: 2}
FILL.update({k: 1 for k in range(1, 16)})

# input DMA chunks: parallel DMA engines may complete out of order even
# within one queue, so every chunk gets its OWN semaphore (wait >= 16).
SYNC_CHUNKS = ['xj00', 'e3', 'x3', 'xj10', 'xi20']
SCAL_CHUNKS = ['xi00', 'xi10', 'w', 'xj01', 'xj11', 'xi21']
GPS_CHUNKS = ['xi01', 'xi11']   # third (SWDGE) queue, idle after memset


def _build_tables(idx1, idx2, out_idx, cb_vals):
    idx1 = np.asarray(idx1, np.int64)
    idx2 = np.asarray(idx2, np.int64)
    out_idx = np.asarray(out_idx, np.int64)
    cb = np.asarray(cb_vals, np.float64)

    offs, blocks = 0, []
    for l in LS:
        blocks.append(list(range(offs, offs + 2 * l + 1)))
        offs += 2 * l + 1
    isets = [blocks[0] + blocks[3], blocks[1] + blocks[2],
             blocks[4] + blocks[7], blocks[5] + blocks[6]]
    jsets = [list(range(16)), list(range(16, 32))]
    imap = {c: (a, il) for a, s in enumerate(isets) for il, c in enumerate(s)}
    jmap = {c: (b, jl) for b, s in enumerate(jsets) for jl, c in enumerate(s)}

    out_q = {}
    for k in range(len(cb)):
        a, _ = imap[int(idx1[k])]
        b, _ = jmap[int(idx2[k])]
        q = a * 2 + b
        o = int(out_idx[k])
        assert out_q.setdefault(o, q) == q
    rows_map = np.zeros(8 * 128, np.int64)
    out_local = {}
    for q in range(8):
        outs = sorted(o for o, qq in out_q.items() if qq == q)
        assert len(outs) == 128
        for m, o in enumerate(outs):
            out_local[o] = m
            rows_map[q * 128 + m] = o

    e3 = np.zeros((8, 128), np.float32)
    for p in range(128):
        e3[p // 16, p] = 1.0

    w = np.zeros((128, 8 * 128), np.float64)
    for k in range(len(cb)):
        a, il = imap[int(idx1[k])]
        b, jl = jmap[int(idx2[k])]
        q = a * 2 + b
        p = il * 16 + jl
        m = out_local[int(out_idx[k])]
        w[p, q * 128 + m] += cb[k]

    iperm = np.concatenate([np.asarray(s) for s in isets])
    bf = ml_dtypes.bfloat16
    return iperm, e3.astype(bf), w.astype(np.float32).astype(bf), rows_map


def _build_bass():
    nc = bacc.Bacc("TRN2", target_bir_lowering=False)

    x3h = nc.dram_tensor("x3h", [8, BC], BF16, kind="ExternalInput")
    xih = nc.dram_tensor("xih", [128, 3 * BC], BF16, kind="ExternalInput")
    xjh = nc.dram_tensor("xjh", [128, 2 * BC], BF16, kind="ExternalInput")
    e3h = nc.dram_tensor("e3h", [8, 128], BF16, kind="ExternalInput")
    wgt = nc.dram_tensor("wgt", [128, 8 * 128], BF16, kind="ExternalInput")
    outT = nc.dram_tensor("outT", [8 * 128, BC], BF16, kind="ExternalOutput")

    es = ExitStack()
    sIN = {k: es.enter_context(nc.semaphore(f"sIN_{k}"))
           for k in SYNC_CHUNKS + SCAL_CHUNKS + GPS_CHUNKS}
    sGPS = es.enter_context(nc.semaphore("sGPS"))
    sPE = es.enter_context(nc.semaphore("sPE"))
    sACT = es.enter_context(nc.semaphore("sACT"))
    sDVE = es.enter_context(nc.semaphore("sDVE"))
    sOUT = es.enter_context(nc.semaphore("sOUT"))

    e3_sb = es.enter_context(nc.sbuf_tensor("e3_sb", [8, 128], BF16))
    w_sb = es.enter_context(nc.sbuf_tensor("w_sb", [128, 8 * 128], BF16))
    wm_sb = es.enter_context(nc.sbuf_tensor("wm_sb", [128, CH], BF16))
    x3_sb = es.enter_context(nc.sbuf_tensor("x3_sb", [8, BC], BF16))
    xi_sb = es.enter_context(nc.sbuf_tensor("xi_sb", [128, 3 * BC], BF16))
    xj_sb = es.enter_context(nc.sbuf_tensor("xj_sb", [128, 2 * BC], BF16))
    r_sb = es.enter_context(nc.sbuf_tensor("r_sb", [128, BC], BF16))
    u_sb = es.enter_context(nc.sbuf_tensor("u_sb", [128, 8 * BC], BF16))
    o_sb = es.enter_context(nc.sbuf_tensor("o_sb", [128, 8 * BC], BF16))
    ps = es.enter_context(nc.psum_tensor("ps", [128, 8 * CH], F32))

    # per-engine high-water marks to skip redundant waits
    hw = {}

    def wait(engine, ename, sem, val):
        key = (ename, sem.num)
        if hw.get(key, 0) < val:
            hw[key] = val
            engine.wait_ge(sem, val)

    # ---- input DMAs -------------------------------------------------
    def ichunk(t_sb, t_h, slab, h):
        lo = slab * BC + h * H
        return t_sb[:, lo : lo + H], t_h[:, lo : lo + H]

    chunks = {
        'e3': (e3_sb[:, :], e3h[:, :]),
        'x3': (x3_sb[:, :], x3h[:, :]),
        'xj00': ichunk(xj_sb, xjh, 0, 0), 'xj10': ichunk(xj_sb, xjh, 1, 0),
        'xi01': ichunk(xi_sb, xih, 0, 1), 'xi11': ichunk(xi_sb, xih, 1, 1),
        'xi20': ichunk(xi_sb, xih, 2, 0),
        'xi00': ichunk(xi_sb, xih, 0, 0), 'xi10': ichunk(xi_sb, xih, 1, 0),
        'w': (w_sb[:, :], wgt[:, :]),
        'xj01': ichunk(xj_sb, xjh, 0, 1), 'xj11': ichunk(xj_sb, xjh, 1, 1),
        'xi21': ichunk(xi_sb, xih, 2, 1),
    }
    for k in SYNC_CHUNKS:
        dst, src = chunks[k]
        nc.sync.dma_start(out=dst, in_=src).then_inc(sIN[k], 16)
    for k in SCAL_CHUNKS:
        dst, src = chunks[k]
        nc.scalar.dma_start(out=dst, in_=src).then_inc(sIN[k], 16)
    for k in GPS_CHUNKS:
        dst, src = chunks[k]
        nc.gpsimd.dma_start(out=dst, in_=src).then_inc(sIN[k], 16)

    # ---- warmup -----------------------------------------------------
    nc.gpsimd.memset(wm_sb[:, :], 0.0).then_inc(sGPS, 1)

    def filler(n):
        for i in range(n):
            j = 6 + (i % 2)
            nc.tensor.matmul(
                ps[:, j * CH : (j + 1) * CH],
                lhsT=wm_sb[:, :128], rhs=wm_sb[:, :],
                start=True, stop=True,
            )

    wait(nc.tensor, 'pe', sGPS, 1)
    filler(N_WARM)

    # ---- PE stream --------------------------------------------------
    # R3 expansion: slices 0-3, sPE 1..4
    wait(nc.tensor, 'pe', sIN['e3'], 16)
    wait(nc.tensor, 'pe', sIN['x3'], 16)
    for c in range(4):
        nc.tensor.matmul(
            ps[:, c * CH : (c + 1) * CH],
            lhsT=e3_sb[:, :], rhs=x3_sb[:, c * CH : (c + 1) * CH],
            start=True, stop=True,
        ).then_inc(sPE, 1)

    # ACT emits R3 casts first (ids 1,2); interleaved below via streams.
    # We emit whole per-engine streams in order; cross-engine order is
    # irrelevant (semaphores sequence everything).

    # PE main items: item k -> slices (4+2k)%6, sPE ids 5+2k, 6+2k
    # WAR: item k wants cast of item k-3 (ACT id 3+#a-items<=k-3, or DVE)
    acast_id = {}
    n_a = 0
    for k, (q, h) in enumerate(M_ORDER):
        if O_CAST[(q, h)] == 'a':
            n_a += 1
            acast_id[k] = 2 + n_a          # after the 2 R3 casts
    # sDVE ids follow DVE_PROG emission order
    mul_id = {}
    dcast_id = {}
    for pos, (kind, idx) in enumerate(DVE_PROG, start=1):
        if kind == 'm':
            mul_id[idx] = pos
        else:
            dcast_id[idx] = pos

    wait(nc.tensor, 'pe', sIN['w'], 16)
    for k, (q, h) in enumerate(M_ORDER):
        filler(FILL.get(k, 0))
        # operand ready (this item's mul done)
        wait(nc.tensor, 'pe', sDVE, mul_id[k])
        # PSUM slice WAR
        if k == 1:
            wait(nc.tensor, 'pe', sACT, 1)
        elif k == 2:
            wait(nc.tensor, 'pe', sACT, 2)
        elif k >= 3:
            kk = k - 3
            if kk in acast_id:
                wait(nc.tensor, 'pe', sACT, acast_id[kk])
            else:
                wait(nc.tensor, 'pe', sDVE, dcast_id[kk])
        i0 = (4 + 2 * k) % 6
        for ci in range(2):
            c = h * 2 + ci
            nc.tensor.matmul(
                ps[:, (i0 + ci) * CH : (i0 + ci + 1) * CH],
                lhsT=w_sb[:, q * 128 : (q + 1) * 128],
                rhs=u_sb[:, q * BC + c * CH : q * BC + (c + 1) * CH],
                start=True, stop=True,
            ).then_inc(sPE, 1)

    # ---- ACT stream -------------------------------------------------
    wait(nc.scalar, 'act', sPE, 2)
    nc.scalar.copy(out=r_sb[:, 0:H], in_=ps[:, 0:H]).then_inc(sACT, 1)
    wait(nc.scalar, 'act', sPE, 4)
    nc.scalar.copy(out=r_sb[:, H:BC], in_=ps[:, H : 2 * H]).then_inc(sACT, 1)
    for k, (q, h) in enumerate(M_ORDER):
        if O_CAST[(q, h)] != 'a':
            continue
        wait(nc.scalar, 'act', sPE, 6 + 2 * k)
        i0 = (4 + 2 * k) % 6
        nc.scalar.copy(
            out=o_sb[:, q * BC + h * H : q * BC + (h + 1) * H],
            in_=ps[:, i0 * CH : (i0 + 2) * CH],
        ).then_inc(sACT, 1)

    # ---- DVE stream -------------------------------------------------
    def mul_waits(q, h):
        a, b = Q_AB[q]
        ws = []
        if a < 3:
            ws.append((sIN[f'xi{a}{h}'], 16))
        else:
            ws.append((sACT, 1 + h))
        ws.append((sIN[f'xj{b}{h}'], 16))
        return ws

    for kind, idx in DVE_PROG:
        if kind == 'm':
            q, h = M_ORDER[idx]
            a, b = Q_AB[q]
            for sem, val in mul_waits(q, h):
                wait(nc.vector, 'dve', sem, val)
            src0 = (xi_sb[:, a * BC + h * H : a * BC + (h + 1) * H]
                    if a < 3 else r_sb[:, h * H : (h + 1) * H])
            nc.vector.tensor_mul(
                u_sb[:, q * BC + h * H : q * BC + (h + 1) * H],
                src0,
                xj_sb[:, b * BC + h * H : b * BC + (h + 1) * H],
            ).then_inc(sDVE, 1)
        else:
            k = idx
            q, h = M_ORDER[k]
            wait(nc.vector, 'dve', sPE, 6 + 2 * k)
            i0 = (4 + 2 * k) % 6
            nc.vector.tensor_copy(
                o_sb[:, q * BC + h * H : q * BC + (h + 1) * H],
                ps[:, i0 * CH : (i0 + 2) * CH],
            ).then_inc(sDVE, 1)

    # ---- output DMAs (sync queue) -----------------------------------
    for k, (q, h) in enumerate(M_ORDER):
        if O_CAST[(q, h)] == 'a':
            wait(nc.sync, 'sync', sACT, acast_id[k])
        else:
            wait(nc.sync, 'sync', sDVE, dcast_id[k])
        nc.sync.dma_start(
            out=outT[q * 128 : (q + 1) * 128, h * H : (h + 1) * H],
            in_=o_sb[:, q * BC + h * H : q * BC + (h + 1) * H],
        ).then_inc(sOUT, 16)
    nc.sync.wait_ge(sOUT, 16 * 16)

    # ---- end-of-program hygiene ------------------------------------
    # The runtime NEFF epilogue wipes the full semaphore space on every
    # execution (observed in all traces), so per-sem clears here would be
    # redundant; one barrier joins the engines before that epilogue.
    nc.all_engine_barrier()

    es.close()
    nc.compile()
    return nc


_CACHE = {}


def kernel(in1, in2, cb_vals, idx1, idx2, out_idx):
    in1 = np.ascontiguousarray(np.asarray(in1, np.float32))
    in2 = np.ascontiguousarray(np.asarray(in2, np.float32))

    key = (
        np.asarray(idx1).tobytes(),
        np.asarray(idx2).tobytes(),
        np.asarray(out_idx).tobytes(),
        np.asarray(cb_vals).tobytes(),
    )
    kh = hash(key)
    if kh not in _CACHE:
        iperm, e3, w, rows_map = _build_tables(idx1, idx2, out_idx, cb_vals)
        nc = _build_bass()
        _CACHE[kh] = (nc, iperm, e3, w, rows_map)
    nc, iperm, e3, w, rows_map = _CACHE[kh]

    bf = ml_dtypes.bfloat16
    in1p = in1[:, iperm].astype(bf)
    in2b = in2.astype(bf)
    in_maps = []
    for core in range(N_CORES):
        sl = slice(core * BC, (core + 1) * BC)
        in1T = np.ascontiguousarray(in1p[sl].T)
        in2T = np.ascontiguousarray(in2b[sl].T)
        x3h = np.ascontiguousarray(in1T[24:32])
        xih = np.ascontiguousarray(np.concatenate(
            [np.repeat(in1T[a * 8 : (a + 1) * 8], 16, axis=0)
             for a in range(3)], axis=1))
        xjh = np.ascontiguousarray(np.concatenate(
            [np.tile(in2T[b * 16 : (b + 1) * 16], (8, 1))
             for b in range(2)], axis=1))
        in_maps.append(
            {"x3h": x3h, "xih": xih, "xjh": xjh, "e3h": e3, "wgt": w}
        )

    trace = bool(int(os.environ.get("KERNEL_TRACE", "0")))
    res = run_bass_kernel_spmd(
        nc, in_maps, core_ids=list(range(N_CORES)), trace=trace
    )
    kernel.last_results = res

    out = np.empty((B, DOUT), np.float32)
    for core in range(N_CORES):
        shard = res.results[core]["outT"]
        out[core * BC : (core + 1) * BC][:, rows_map] = (
            np.asarray(shard).astype(np.float32).T
        )
    return out


# revision 28
# speedup vs baseline: 1.0917x; 1.0917x over previous
"""Raw-Bass (no TileContext) version of the CudaTensorProduct kernel.

Same v5 pipeline as kernel.py, but with hand-rolled counting semaphores so
the tile framework's fixed ~10us teardown (254 single-semaphore clears +
drain ceremony) is replaced by one barrier + a handful of clears.

Counting semaphores (all monotone within one execution):
  sIN[chunk]    : +16 at each input-DMA chunk's completion (one semaphore
                  per chunk: the 16 parallel DMA engines can complete
                  transfers out of order even within one queue)
  sGPS          : +1 after the warmup-tile memset
  sPE           : +1 per REAL PE matmul (R3: 1..4, main item k: 5+2k, 6+2k)
  sACT          : +1 per ACT cast (R3 halves: 1,2; then out-casts 3..14)
  sDVE          : +1 per DVE op (muls 1..16, tile-4/5 out-casts 17..20)
  sOUT          : +16 per output-DMA completion (final drain waits 256)
"""

import os
import sys
from contextlib import ExitStack

import numpy as np
import ml_dtypes

sys.path.insert(0, "/opt/trn_rl_repo")

import concourse.bass as bass
import concourse.mybir as mybir
from concourse import bacc
from concourse.bass_utils import run_bass_kernel_spmd

N_CORES = 8
B = 16384
BC = B // N_CORES
D1 = 32
DOUT = 1024
F32 = mybir.dt.float32
BF16 = mybir.dt.bfloat16
LS = [0, 1, 2, 3, 0, 1, 2, 3]

N_WARM = 10
CH = 512
H = 1024

# main/mul item order (q, h); identical for both streams
M_ORDER = [(0, 0), (1, 0), (2, 0), (3, 0), (6, 0), (0, 1),
           (7, 0), (1, 1), (2, 1), (3, 1), (6, 1), (7, 1),
           (4, 0), (5, 0), (4, 1), (5, 1)]
Q_AB = {q: (q // 2, q % 2) for q in range(8)}
O_CAST = {(q, h): 'a' for q in range(8) for h in range(2)}
O_CAST.update({(4, 0): 'v', (4, 1): 'v', (5, 0): 'v', (5, 1): 'v'})
# DVE emission program: all 16 muls first, then the tile-4/5 out-casts
# (interleaving casts earlier stalls the DVE queue on PE progress).
DVE_PROG = ([('m', i) for i in range(16)]
            + [('c', 12), ('c', 13), ('c', 14), ('c', 15)])
# fillers emitted before main item k
FILL = {0: 2}
FILL.update({k: 1 for k in range(1, 16)})

# input DMA chunks: parallel DMA engines may complete out of order even
# within one queue, so every chunk gets its OWN semaphore (wait >= 16).
SYNC_CHUNKS = ['xj00', 'e3', 'x3', 'xj10', 'xi01', 'xi11', 'xi20']
SCAL_CHUNKS = ['xi00', 'xi10', 'w', 'xj01', 'xj11', 'xi21']
GPS_CHUNKS = []  # SWDGE input path measured slower (fixed ~1us overhead)


def _build_tables(idx1, idx2, out_idx, cb_vals):
    idx1 = np.asarray(idx1, np.int64)
    idx2 = np.asarray(idx2, np.int64)
    out_idx = np.asarray(out_idx, np.int64)
    cb = np.asarray(cb_vals, np.float64)

    offs, blocks = 0, []
    for l in LS:
        blocks.append(list(range(offs, offs + 2 * l + 1)))
        offs += 2 * l + 1
    isets = [blocks[0] + blocks[3], blocks[1] + blocks[2],
             blocks[4] + blocks[7], blocks[5] + blocks[6]]
    jsets = [list(range(16)), list(range(16, 32))]
    imap = {c: (a, il) for a, s in enumerate(isets) for il, c in enumerate(s)}
    jmap = {c: (b, jl) for b, s in enumerate(jsets) for jl, c in enumerate(s)}

    out_q = {}
    for k in range(len(cb)):
        a, _ = imap[int(idx1[k])]
        b, _ = jmap[int(idx2[k])]
        q = a * 2 + b
        o = int(out_idx[k])
        assert out_q.setdefault(o, q) == q
    rows_map = np.zeros(8 * 128, np.int64)
    out_local = {}
    for q in range(8):
        outs = sorted(o for o, qq in out_q.items() if qq == q)
        assert len(outs) == 128
        for m, o in enumerate(outs):
            out_local[o] = m
            rows_map[q * 128 + m] = o

    e3 = np.zeros((8, 128), np.float32)
    for p in range(128):
        e3[p // 16, p] = 1.0

    w = np.zeros((128, 8 * 128), np.float64)
    for k in range(len(cb)):
        a, il = imap[int(idx1[k])]
        b, jl = jmap[int(idx2[k])]
        q = a * 2 + b
        p = il * 16 + jl
        m = out_local[int(out_idx[k])]
        w[p, q * 128 + m] += cb[k]

    iperm = np.concatenate([np.asarray(s) for s in isets])
    bf = ml_dtypes.bfloat16
    return iperm, e3.astype(bf), w.astype(np.float32).astype(bf), rows_map


def _build_bass():
    nc = bacc.Bacc("TRN2", target_bir_lowering=False)

    x3h = nc.dram_tensor("x3h", [8, BC], BF16, kind="ExternalInput")
    xih = nc.dram_tensor("xih", [128, 3 * BC], BF16, kind="ExternalInput")
    xjh = nc.dram_tensor("xjh", [128, 2 * BC], BF16, kind="ExternalInput")
    e3h = nc.dram_tensor("e3h", [8, 128], BF16, kind="ExternalInput")
    wgt = nc.dram_tensor("wgt", [128, 8 * 128], BF16, kind="ExternalInput")
    outT = nc.dram_tensor("outT", [8 * 128, BC], BF16, kind="ExternalOutput")

    es = ExitStack()
    sIN = {k: es.enter_context(nc.semaphore(f"sIN_{k}"))
           for k in SYNC_CHUNKS + SCAL_CHUNKS + GPS_CHUNKS}
    sGPS = es.enter_context(nc.semaphore("sGPS"))
    sPE = es.enter_context(nc.semaphore("sPE"))
    sACT = es.enter_context(nc.semaphore("sACT"))
    sDVE = es.enter_context(nc.semaphore("sDVE"))
    sOUT = es.enter_context(nc.semaphore("sOUT"))

    e3_sb = es.enter_context(nc.sbuf_tensor("e3_sb", [8, 128], BF16))
    w_sb = es.enter_context(nc.sbuf_tensor("w_sb", [128, 8 * 128], BF16))
    wm_sb = es.enter_context(nc.sbuf_tensor("wm_sb", [128, CH], BF16))
    x3_sb = es.enter_context(nc.sbuf_tensor("x3_sb", [8, BC], BF16))
    xi_sb = es.enter_context(nc.sbuf_tensor("xi_sb", [128, 3 * BC], BF16))
    xj_sb = es.enter_context(nc.sbuf_tensor("xj_sb", [128, 2 * BC], BF16))
    r_sb = es.enter_context(nc.sbuf_tensor("r_sb", [128, BC], BF16))
    u_sb = es.enter_context(nc.sbuf_tensor("u_sb", [128, 8 * BC], BF16))
    o_sb = es.enter_context(nc.sbuf_tensor("o_sb", [128, 8 * BC], BF16))
    ps = es.enter_context(nc.psum_tensor("ps", [128, 8 * CH], F32))

    # per-engine high-water marks to skip redundant waits
    hw = {}

    def wait(engine, ename, sem, val):
        key = (ename, sem.num)
        if hw.get(key, 0) < val:
            hw[key] = val
            engine.wait_ge(sem, val)

    # ---- input DMAs -------------------------------------------------
    def ichunk(t_sb, t_h, slab, h):
        lo = slab * BC + h * H
        return t_sb[:, lo : lo + H], t_h[:, lo : lo + H]

    chunks = {
        'e3': (e3_sb[:, :], e3h[:, :]),
        'x3': (x3_sb[:, :], x3h[:, :]),
        'xj00': ichunk(xj_sb, xjh, 0, 0), 'xj10': ichunk(xj_sb, xjh, 1, 0),
        'xi01': ichunk(xi_sb, xih, 0, 1), 'xi11': ichunk(xi_sb, xih, 1, 1),
        'xi20': ichunk(xi_sb, xih, 2, 0),
        'xi00': ichunk(xi_sb, xih, 0, 0), 'xi10': ichunk(xi_sb, xih, 1, 0),
        'w': (w_sb[:, :], wgt[:, :]),
        'xj01': ichunk(xj_sb, xjh, 0, 1), 'xj11': ichunk(xj_sb, xjh, 1, 1),
        'xi21': ichunk(xi_sb, xih, 2, 1),
    }
    for k in SYNC_CHUNKS:
        dst, src = chunks[k]
        nc.sync.dma_start(out=dst, in_=src).then_inc(sIN[k], 16)
    for k in SCAL_CHUNKS:
        dst, src = chunks[k]
        nc.scalar.dma_start(out=dst, in_=src).then_inc(sIN[k], 16)
    for k in GPS_CHUNKS:
        dst, src = chunks[k]
        nc.gpsimd.dma_start(out=dst, in_=src).then_inc(sIN[k], 16)

    # ---- warmup -----------------------------------------------------
    nc.gpsimd.memset(wm_sb[:, :], 0.0).then_inc(sGPS, 1)

    def filler(n):
        for i in range(n):
            j = 6 + (i % 2)
            nc.tensor.matmul(
                ps[:, j * CH : (j + 1) * CH],
                lhsT=wm_sb[:, :128], rhs=wm_sb[:, :],
                start=True, stop=True,
            )

    wait(nc.tensor, 'pe', sGPS, 1)
    filler(N_WARM)

    # ---- PE stream --------------------------------------------------
    # R3 expansion: slices 0-3, sPE 1..4
    wait(nc.tensor, 'pe', sIN['e3'], 16)
    wait(nc.tensor, 'pe', sIN['x3'], 16)
    for c in range(4):
        nc.tensor.matmul(
            ps[:, c * CH : (c + 1) * CH],
            lhsT=e3_sb[:, :], rhs=x3_sb[:, c * CH : (c + 1) * CH],
            start=True, stop=True,
        ).then_inc(sPE, 1)

    # ACT emits R3 casts first (ids 1,2); interleaved below via streams.
    # We emit whole per-engine streams in order; cross-engine order is
    # irrelevant (semaphores sequence everything).

    # PE main items: item k -> slices (4+2k)%6, sPE ids 5+2k, 6+2k
    # WAR: item k wants cast of item k-3 (ACT id 3+#a-items<=k-3, or DVE)
    acast_id = {}
    n_a = 0
    for k, (q, h) in enumerate(M_ORDER):
        if O_CAST[(q, h)] == 'a':
            n_a += 1
            acast_id[k] = 2 + n_a          # after the 2 R3 casts
    # sDVE ids follow DVE_PROG emission order
    mul_id = {}
    dcast_id = {}
    for pos, (kind, idx) in enumerate(DVE_PROG, start=1):
        if kind == 'm':
            mul_id[idx] = pos
        else:
            dcast_id[idx] = pos

    wait(nc.tensor, 'pe', sIN['w'], 16)
    for k, (q, h) in enumerate(M_ORDER):
        filler(FILL.get(k, 0))
        # operand ready (this item's mul done)
        wait(nc.tensor, 'pe', sDVE, mul_id[k])
        # PSUM slice WAR
        if k == 1:
            wait(nc.tensor, 'pe', sACT, 1)
        elif k == 2:
            wait(nc.tensor, 'pe', sACT, 2)
        elif k >= 3:
            kk = k - 3
            if kk in acast_id:
                wait(nc.tensor, 'pe', sACT, acast_id[kk])
            else:
                wait(nc.tensor, 'pe', sDVE, dcast_id[kk])
        i0 = (4 + 2 * k) % 6
        for ci in range(2):
            c = h * 2 + ci
            nc.tensor.matmul(
                ps[:, (i0 + ci) * CH : (i0 + ci + 1) * CH],
                lhsT=w_sb[:, q * 128 : (q + 1) * 128],
                rhs=u_sb[:, q * BC + c * CH : q * BC + (c + 1) * CH],
                start=True, stop=True,
            ).then_inc(sPE, 1)

    # ---- ACT stream -------------------------------------------------
    wait(nc.scalar, 'act', sPE, 2)
    nc.scalar.copy(out=r_sb[:, 0:H], in_=ps[:, 0:H]).then_inc(sACT, 1)
    wait(nc.scalar, 'act', sPE, 4)
    nc.scalar.copy(out=r_sb[:, H:BC], in_=ps[:, H : 2 * H]).then_inc(sACT, 1)
    for k, (q, h) in enumerate(M_ORDER):
        if O_CAST[(q, h)] != 'a':
            continue
        wait(nc.scalar, 'act', sPE, 6 + 2 * k)
        i0 = (4 + 2 * k) % 6
        nc.scalar.copy(
            out=o_sb[:, q * BC + h * H : q * BC + (h + 1) * H],
            in_=ps[:, i0 * CH : (i0 + 2) * CH],
        ).then_inc(sACT, 1)

    # ---- DVE stream -------------------------------------------------
    def mul_waits(q, h):
        a, b = Q_AB[q]
        ws = []
        if a < 3:
            ws.append((sIN[f'xi{a}{h}'], 16))
        else:
            ws.append((sACT, 1 + h))
        ws.append((sIN[f'xj{b}{h}'], 16))
        return ws

    for kind, idx in DVE_PROG:
        if kind == 'm':
            q, h = M_ORDER[idx]
            a, b = Q_AB[q]
            for sem, val in mul_waits(q, h):
                wait(nc.vector, 'dve', sem, val)
            src0 = (xi_sb[:, a * BC + h * H : a * BC + (h + 1) * H]
                    if a < 3 else r_sb[:, h * H : (h + 1) * H])
            nc.vector.tensor_mul(
                u_sb[:, q * BC + h * H : q * BC + (h + 1) * H],
                src0,
                xj_sb[:, b * BC + h * H : b * BC + (h + 1) * H],
            ).then_inc(sDVE, 1)
        else:
            k = idx
            q, h = M_ORDER[k]
            wait(nc.vector, 'dve', sPE, 6 + 2 * k)
            i0 = (4 + 2 * k) % 6
            nc.vector.tensor_copy(
                o_sb[:, q * BC + h * H : q * BC + (h + 1) * H],
                ps[:, i0 * CH : (i0 + 2) * CH],
            ).then_inc(sDVE, 1)

    # ---- output DMAs (sync queue) -----------------------------------
    for k, (q, h) in enumerate(M_ORDER):
        if O_CAST[(q, h)] == 'a':
            wait(nc.sync, 'sync', sACT, acast_id[k])
        else:
            wait(nc.sync, 'sync', sDVE, dcast_id[k])
        nc.sync.dma_start(
            out=outT[q * 128 : (q + 1) * 128, h * H : (h + 1) * H],
            in_=o_sb[:, q * BC + h * H : q * BC + (h + 1) * H],
        ).then_inc(sOUT, 16)
    nc.sync.wait_ge(sOUT, 16 * 16)

    # ---- end-of-program hygiene ------------------------------------
    # The runtime NEFF epilogue wipes the full semaphore space on every
    # execution (observed in all traces), so per-sem clears here would be
    # redundant; one barrier joins the engines before that epilogue.
    nc.all_engine_barrier()

    es.close()
    nc.compile()
    return nc


_CACHE = {}


def kernel(in1, in2, cb_vals, idx1, idx2, out_idx):
    in1 = np.ascontiguousarray(np.asarray(in1, np.float32))
    in2 = np.ascontiguousarray(np.asarray(in2, np.float32))

    key = (
        np.asarray(idx1).tobytes(),
        np.asarray(idx2).tobytes(),
        np.asarray(out_idx).tobytes(),
        np.asarray(cb_vals).tobytes(),
    )
    kh = hash(key)
    if kh not in _CACHE:
        iperm, e3, w, rows_map = _build_tables(idx1, idx2, out_idx, cb_vals)
        nc = _build_bass()
        _CACHE[kh] = (nc, iperm, e3, w, rows_map)
    nc, iperm, e3, w, rows_map = _CACHE[kh]

    bf = ml_dtypes.bfloat16
    in1p = in1[:, iperm].astype(bf)
    in2b = in2.astype(bf)
    in_maps = []
    for core in range(N_CORES):
        sl = slice(core * BC, (core + 1) * BC)
        in1T = np.ascontiguousarray(in1p[sl].T)
        in2T = np.ascontiguousarray(in2b[sl].T)
        x3h = np.ascontiguousarray(in1T[24:32])
        xih = np.ascontiguousarray(np.concatenate(
            [np.repeat(in1T[a * 8 : (a + 1) * 8], 16, axis=0)
             for a in range(3)], axis=1))
        xjh = np.ascontiguousarray(np.concatenate(
            [np.tile(in2T[b * 16 : (b + 1) * 16], (8, 1))
             for b in range(2)], axis=1))
        in_maps.append(
            {"x3h": x3h, "xih": xih, "xjh": xjh, "e3h": e3, "wgt": w}
        )

    trace = bool(int(os.environ.get("KERNEL_TRACE", "0")))
    res = run_bass_kernel_spmd(
        nc, in_maps, core_ids=list(range(N_CORES)), trace=trace
    )
    kernel.last_results = res

    out = np.empty((B, DOUT), np.float32)
    for core in range(N_CORES):
        shard = res.results[core]["outT"]
        out[core * BC : (core + 1) * BC][:, rows_map] = (
            np.asarray(shard).astype(np.float32).T
        )
    return out
